# revision 2
# baseline (speedup 1.0000x reference)
"""CQT magnitude kernel for Trainium2, distributed over 8 NeuronCores.

Problem: |CQT| of a 5 s, 44.1 kHz signal. Output [252 bins, 456 frames].

The reference materializes frames [456, 69364] (hop 484, centered) and runs
two dense matmuls against a [252, 69364] complex kernel bank, then takes the
magnitude. That bank is ~80% zeros: per-bin Hann windows are centered at
CENTER with constant-Q lengths that shrink geometrically from 69364 down to
~553 samples.

Strategy used here:
 - Never materialize frames. Since HOP = 484 = 4*121, the transposed frame
   matrix needed as the matmul moving operand is a column-slice of a small
   "phase-split" signal matrix SigT[phi][k, i] = padded[484*i + 121*phi + k]
   (k in [0,121)), staged on the host. Each K-chunk of 121 samples of every
   window offset is then rhs = SigT[phi][:, q : q+456].
 - Pack the (re, im) kernel bank rows interleaved per bin into 4 row-blocks
   of 64 bins (128 PSUM rows), each with its K-support trimmed to the
   longest bin in the block, and per-chunk column counts trimmed to the
   set of bins actually alive in that K-chunk (valid because bin lengths
   are sorted, so alive bins are always a prefix of the block's rows).
 - Split the K-chunks of every block across the 8 cores round-robin
   (global chunk j = 8*j' + core). Every core runs the identical program;
   per-core behavior differs only in staged input data (the signal window
   is rotated by `core` columns, kernel slabs hold the core's chunks).
   Each core accumulates partial sums for all 4 blocks in PSUM and writes
   a [512, 456] partial; the host reduces over cores, de-interleaves
   re/im rows and takes the magnitude.
"""

import math
import os
import sys
from functools import lru_cache

import numpy as np

for _p in ("/opt/trn_rl_repo", "/root/.axon_site/_ro/trn_rl_repo"):
    if os.path.isdir(_p) and _p not in sys.path:
        sys.path.insert(0, _p)

# ----------------------------------------------------------------------------
# CQT geometry (must match the reference exactly)
# ----------------------------------------------------------------------------
SR = 44100
BPO = 36
N_BINS = 252
HOP = 484
FMIN = 32.70319566257483
Q = 1.0 / (2.0 ** (1.0 / BPO) - 1.0)
FREQS = FMIN * 2.0 ** (np.arange(N_BINS) / BPO)  # float64
LENGTHS = Q * SR / FREQS  # float64, strictly decreasing
MAX_LEN = int(np.ceil(LENGTHS[0]))  # 69364
CENTER = MAX_LEN // 2  # 34682
SIG_LEN = 220500
N_FRAMES = 1 + SIG_LEN // HOP  # 456

NCORES = 8
KCH = 121  # K per matmul chunk; HOP = 4 * KCH
NPHASE = 4
N_SIGCOLS = 600  # columns of SigT; covers padded signal
PAD_LEN = HOP * N_SIGCOLS  # 290400 >= SIG_LEN + MAX_LEN

# Row-blocks: 64 bins each (128 rows: re/im interleaved per bin)
_BIN0S = [0, 64, 128, 192]
_J8S = [18, 6, 2, 1]  # per-core chunk count per phase per block


def _block_geom():
    blocks = []
    for b in range(4):
        bin0 = _BIN0S[b]
        nbins = min(64, N_BINS - bin0)
        lmax = LENGTHS[bin0]
        lo = CENTER - lmax / 2.0
        t0 = max(0, int(math.floor(lo / HOP))) * HOP
        j8 = _J8S[b]
        jtot = NCORES * j8
        # k-range [t0, t0 + HOP*jtot) must cover the block's support
        assert t0 <= lo or t0 == 0, (b, t0, lo)  # support below t=0 doesn't exist
        assert t0 + HOP * jtot >= CENTER + lmax / 2.0, (b, t0 + HOP * jtot)
        blocks.append(dict(bin0=bin0, nbins=nbins, t0=t0, qb=t0 // HOP, j8=j8))
    return blocks


BLOCKS = _block_geom()
# Signal window width per phase (core-independent slice offsets QB + 8*j')
W_SIG = N_FRAMES + max(blk["qb"] + 8 * (blk["j8"] - 1) for blk in BLOCKS)
assert W_SIG + (NCORES - 1) <= N_SIGCOLS, W_SIG


def _alive_count(b, k0):
    """Number of bins of block b whose window support intersects [k0, k0+KCH)."""
    blk = BLOCKS[b]
    if k0 + KCH - 1 < CENTER:
        d = CENTER - (k0 + KCH - 1)
    elif k0 > CENTER:
        d = k0 - CENTER
    else:
        d = 0
    lens = LENGTHS[blk["bin0"] : blk["bin0"] + blk["nbins"]]
    return int(np.sum(lens > 2.0 * d))


def _chunk_schedule():
    """Chunk list per block, in device emission order, plus kin slab layout.

    Every chunk: dict(b, phi, jp, m, sigcol, moff, slab). The same schedule
    drives host data staging and trace-time program emission, so the two
    always agree. Identical across cores by construction (m maxes over c).
    """
    per_block = []
    for b in range(4):
        blk = BLOCKS[b]
        chunks = []
        for phi in range(NPHASE):
            grp = []
            for jp in range(blk["j8"]):
                m2 = max(
                    _alive_count(b, blk["t0"] + HOP * (8 * jp + c) + KCH * phi)
                    for c in range(NCORES)
                )
                if m2 == 0:
                    continue
                grp.append(
                    dict(b=b, phi=phi, jp=jp, m=2 * m2, sigcol=blk["qb"] + 8 * jp)
                )
            grp.sort(key=lambda g: -g["m"])
            chunks.extend(grp)
        assert chunks and chunks[0]["m"] == max(ch["m"] for ch in chunks)
        per_block.append(chunks)

    # kin column offsets in global emission order + slab grouping for DMA
    slabs = []  # (name, col0, width)
    moff = 0
    for b in range(4):
        chunks = per_block[b]
        if b == 0:
            # split each phase group in two for finer DMA pipelining
            groups = []
            for phi in range(NPHASE):
                g = [ch for ch in chunks if ch["phi"] == phi]
                h = (len(g) + 1) // 2
                groups.append((f"b0p{phi}a", g[:h]))
                groups.append((f"b0p{phi}b", g[h:]))
        elif b == 1:
            groups = [
                (f"b1p{phi}", [ch for ch in chunks if ch["phi"] == phi])
                for phi in range(NPHASE)
            ]
        else:
            groups = [(f"b{b}", chunks)]
        for name, g in groups:
            col0 = moff
            for ch in g:
                ch["moff"] = moff
                ch["slab"] = name
                moff += ch["m"]
            slabs.append((name, col0, moff - col0))
    return per_block, slabs, moff


CHUNKS_PER_BLOCK, KIN_SLABS, KIN_COLS = _chunk_schedule()
M0S = [chunks[0]["m"] for chunks in CHUNKS_PER_BLOCK]  # rows written per block
OUT_ROWS = 512  # 4 blocks x 128 rows


# ----------------------------------------------------------------------------
# Host staging
# ----------------------------------------------------------------------------
def _cqt_kernel_bank():
    """Interleaved (re, im) kernel bank, float32 [2*N_BINS, KW].

    Row 2*b is kr[bin b], row 2*b+1 is ki[bin b]. Computed in float64 with
    the reference's formulas (Hann window of per-bin constant-Q length,
    L1-normalized, 1/sqrt(L) scaling).
    """
    kw = max(blk["t0"] + HOP * NCORES * blk["j8"] for blk in BLOCKS)
    bank = np.zeros((2 * N_BINS, kw), dtype=np.float32)
    t = np.arange(kw, dtype=np.float64) - CENTER
    two_pi = 2.0 * np.pi
    step = 36
    for s in range(0, N_BINS, step):
        e = min(s + step, N_BINS)
        freqs = FREQS[s:e, None]
        lens = LENGTHS[s:e, None]
        win = np.where(
            np.abs(t)[None, :] < lens / 2.0,
            0.5 * (1.0 + np.cos(two_pi * t[None, :] / lens)),
            0.0,
        )
        ang = two_pi * freqs * t[None, :] / SR
        norm = win.sum(axis=-1, keepdims=True) * np.sqrt(lens)
        bank[2 * s : 2 * e : 2] = (win * np.cos(ang) / norm).astype(np.float32)
        bank[2 * s + 1 : 2 * e : 2] = (win * np.sin(ang) / norm).astype(np.float32)
    return bank


def _stage_inputs(signal):
    """Build per-core {sig, kin} input maps."""
    signal = np.asarray(signal, dtype=np.float32).reshape(SIG_LEN)
    padded = np.zeros(PAD_LEN, dtype=np.float32)
    padded[CENTER : CENTER + SIG_LEN] = signal
    # SigT[phi][k, i] = padded[HOP*i + KCH*phi + k]
    sig_t = np.ascontiguousarray(padded.reshape(N_SIGCOLS, HOP).T)  # [HOP, 600]

    bank = _cqt_kernel_bank()

    in_maps = []
    for c in range(NCORES):
        sig_c = np.empty((KCH, NPHASE * W_SIG), dtype=np.float32)
        for phi in range(NPHASE):
            sig_c[:, phi * W_SIG : (phi + 1) * W_SIG] = sig_t[
                KCH * phi : KCH * (phi + 1), c : c + W_SIG
            ]
        kin_c = np.zeros((KCH, KIN_COLS), dtype=np.float32)
        for chunks in CHUNKS_PER_BLOCK:
            b = chunks[0]["b"]
            blk = BLOCKS[b]
            for ch in chunks:
                k0 = blk["t0"] + HOP * (8 * ch["jp"] + c) + KCH * ch["phi"]
                r0 = 2 * blk["bin0"]
                kin_c[:, ch["moff"] : ch["moff"] + ch["m"]] = bank[
                    r0 : r0 + ch["m"], k0 : k0 + KCH
                ].T
        in_maps.append({"sig": sig_c, "kin": kin_c})
    return in_maps


# ----------------------------------------------------------------------------
# Device program (identical on all 8 cores)
# ----------------------------------------------------------------------------
@lru_cache(maxsize=1)
def _build_program():
    import concourse.bass as bass  # noqa: F401
    import concourse.mybir as mybir
    from concourse import bacc
    from concourse.tile import TileContext

    f32 = mybir.dt.float32
    nc = bacc.Bacc("TRN2", target_bir_lowering=False, debug=False, num_devices=NCORES)
    sig = nc.dram_tensor("sig", [KCH, NPHASE * W_SIG], f32, kind="ExternalInput")
    kin = nc.dram_tensor("kin", [KCH, KIN_COLS], f32, kind="ExternalInput")
    out = nc.dram_tensor("out", [OUT_ROWS, N_FRAMES], f32, kind="ExternalOutput")

    slab_cols = {name: (c0, w) for name, c0, w in KIN_SLABS}

    with TileContext(nc) as tc:
        with (
            tc.tile_pool(name="sigp", bufs=1) as sigp,
            tc.tile_pool(name="kinp", bufs=1) as kinp,
            tc.tile_pool(name="psp", bufs=4, space="PSUM") as psp,
            tc.tile_pool(name="outp", bufs=2) as outp,
        ):
            # Issue DMAs in the order the PE consumes the data: each phase's
            # signal window just before block-0's kin slabs of that phase,
            # then the smaller blocks' slabs.
            sig_tiles = [None] * NPHASE

            def load_sig(phi):
                st = sigp.tile([KCH, W_SIG], f32, tag=f"sig{phi}", name=f"sig{phi}")
                nc.sync.dma_start(st[:], sig[:, phi * W_SIG : (phi + 1) * W_SIG])
                sig_tiles[phi] = st

            slab_tiles = {}

            def load_slab(name):
                c0, w = slab_cols[name]
                t = kinp.tile([KCH, w], f32, tag=f"kin_{name}", name=f"kin_{name}")
                nc.sync.dma_start(t[:], kin[:, c0 : c0 + w])
                slab_tiles[name] = t

            for phi in range(NPHASE):
                load_sig(phi)
                load_slab(f"b0p{phi}a")
                load_slab(f"b0p{phi}b")
            for phi in range(NPHASE):
                load_slab(f"b1p{phi}")
            load_slab("b2")
            load_slab("b3")

            for b in range(4):
                chunks = CHUNKS_PER_BLOCK[b]
                ps = psp.tile([128, N_FRAMES], f32, tag="ps", name=f"ps{b}")
                for i, ch in enumerate(chunks):
                    kt = slab_tiles[ch["slab"]]
                    c0, _ = slab_cols[ch["slab"]]
                    lo = ch["moff"] - c0
                    nc.tensor.matmul(
                        ps[0 : ch["m"], :],
                        kt[:, lo : lo + ch["m"]],
                        sig_tiles[ch["phi"]][:, ch["sigcol"] : ch["sigcol"] + N_FRAMES],
                        start=(i == 0),
                        stop=(i == len(chunks) - 1),
                    )
                m0 = M0S[b]
                ot = outp.tile([m0, N_FRAMES], f32, tag="out", name=f"out{b}")
                nc.vector.tensor_copy(out=ot[:], in_=ps[0:m0, :])
                nc.sync.dma_start(out[128 * b : 128 * b + m0, :], ot[:])
    nc.compile()
    return nc


# ----------------------------------------------------------------------------
# Entry points
# ----------------------------------------------------------------------------
def _run(signal, trace=False, trace_cores=None):
    from concourse import bass_utils

    nc = _build_program()
    in_maps = _stage_inputs(signal)
    res = bass_utils.run_bass_kernel_spmd(
        nc,
        in_maps,
        core_ids=list(range(NCORES)),
        trace=trace,
        trace_cores=trace_cores,
    )
    total = np.zeros((OUT_ROWS, N_FRAMES), dtype=np.float64)
    for c in range(NCORES):
        total += res.results[c]["out"]
    mag = np.empty((N_BINS, N_FRAMES), dtype=np.float64)
    for b, blk in enumerate(BLOCKS):
        rows = total[128 * b : 128 * b + 2 * blk["nbins"]]
        cr = rows[0::2]
        ci = rows[1::2]
        mag[blk["bin0"] : blk["bin0"] + blk["nbins"]] = np.sqrt(cr * cr + ci * ci)
    return mag.astype(np.float32), res


def kernel(signal):
    out, _ = _run(signal)
    return out


# revision 5
# speedup vs baseline: 2.2647x; 2.2647x over previous
"""CQT magnitude kernel for Trainium2, distributed over 8 NeuronCores.

Problem: |CQT| of a 5 s, 44.1 kHz signal. Output [252 bins, 456 frames].

The reference materializes frames [456, 69364] (hop 484, centered) and runs
two dense matmuls against a [252, 69364] complex kernel bank, then takes the
magnitude. That bank is ~80% zeros: per-bin Hann windows are centered at
CENTER with constant-Q lengths that shrink geometrically from 69364 down to
~553 samples.

Strategy used here:
 - Never materialize frames. Since HOP = 484 = 4*121, the transposed frame
   matrix needed as the matmul moving operand is a column-slice of a small
   "phase-split" signal matrix SigT[phi][k, i] = padded[484*i + 121*phi + k]
   (k in [0,121)), staged on the host. Each K-chunk of 121 samples of every
   window offset is then rhs = SigT[phi][:, q : q+456].
 - Pack the (re, im) kernel bank rows interleaved per bin into 4 row-blocks
   of 64 bins (128 PSUM rows), each with its K-support trimmed to the
   longest bin in the block, and per-chunk column counts trimmed to the
   set of bins actually alive in that K-chunk (valid because bin lengths
   are sorted, so alive bins are always a prefix of the block's rows).
 - Split the K-chunks of every block across the 8 cores round-robin
   (global chunk j = 8*j' + core). Every core runs the identical program;
   per-core behavior differs only in staged input data (the signal window
   is rotated by `core` columns, kernel slabs hold the core's chunks).
   Each core accumulates partial sums for all 4 blocks in PSUM and writes
   a [512, 456] partial; the host reduces over cores, de-interleaves
   re/im rows and takes the magnitude.
"""

import math
import os
import sys
from functools import lru_cache

import numpy as np

for _p in ("/opt/trn_rl_repo", "/root/.axon_site/_ro/trn_rl_repo"):
    if os.path.isdir(_p) and _p not in sys.path:
        sys.path.insert(0, _p)

# ----------------------------------------------------------------------------
# CQT geometry (must match the reference exactly)
# ----------------------------------------------------------------------------
SR = 44100
BPO = 36
N_BINS = 252
HOP = 484
FMIN = 32.70319566257483
Q = 1.0 / (2.0 ** (1.0 / BPO) - 1.0)
FREQS = FMIN * 2.0 ** (np.arange(N_BINS) / BPO)  # float64
LENGTHS = Q * SR / FREQS  # float64, strictly decreasing
MAX_LEN = int(np.ceil(LENGTHS[0]))  # 69364
CENTER = MAX_LEN // 2  # 34682
SIG_LEN = 220500
N_FRAMES = 1 + SIG_LEN // HOP  # 456

NCORES = 8
KCH = 121  # K per matmul chunk; HOP = 4 * KCH
NPHASE = 4
N_SIGCOLS = 600  # columns of SigT; covers padded signal
PAD_LEN = HOP * N_SIGCOLS  # 290400 >= SIG_LEN + MAX_LEN

# Row-blocks: 64 bins each (128 rows: re/im interleaved per bin)
_BIN0S = [0, 64, 128, 192]
_J8S = [18, 6, 2, 1]  # per-core chunk count per phase per block


def _block_geom():
    blocks = []
    for b in range(4):
        bin0 = _BIN0S[b]
        nbins = min(64, N_BINS - bin0)
        lmax = LENGTHS[bin0]
        lo = CENTER - lmax / 2.0
        t0 = max(0, int(math.floor(lo / HOP))) * HOP
        j8 = _J8S[b]
        jtot = NCORES * j8
        # k-range [t0, t0 + HOP*jtot) must cover the block's support
        assert t0 <= lo or t0 == 0, (b, t0, lo)  # support below t=0 doesn't exist
        assert t0 + HOP * jtot >= CENTER + lmax / 2.0, (b, t0 + HOP * jtot)
        blocks.append(dict(bin0=bin0, nbins=nbins, t0=t0, qb=t0 // HOP, j8=j8))
    return blocks


BLOCKS = _block_geom()
# Signal window width per phase (core-independent slice offsets QB + 8*j')
W_SIG = N_FRAMES + max(blk["qb"] + 8 * (blk["j8"] - 1) for blk in BLOCKS)
assert W_SIG + (NCORES - 1) <= N_SIGCOLS, W_SIG


def _alive_count(b, k0):
    """Number of bins of block b whose window support intersects [k0, k0+KCH)."""
    blk = BLOCKS[b]
    if k0 + KCH - 1 < CENTER:
        d = CENTER - (k0 + KCH - 1)
    elif k0 > CENTER:
        d = k0 - CENTER
    else:
        d = 0
    lens = LENGTHS[blk["bin0"] : blk["bin0"] + blk["nbins"]]
    return int(np.sum(lens > 2.0 * d))


def _chunk_schedule():
    """Chunk list per block, in device emission order, plus kin slab layout.

    Every chunk: dict(b, phi, jp, m, sigcol, moff, slab). The same schedule
    drives host data staging and trace-time program emission, so the two
    always agree. Identical across cores by construction (m maxes over c).
    """
    per_block = []
    for b in range(4):
        blk = BLOCKS[b]
        chunks = []
        for phi in range(NPHASE):
            grp = []
            for jp in range(blk["j8"]):
                m2 = max(
                    _alive_count(b, blk["t0"] + HOP * (8 * jp + c) + KCH * phi)
                    for c in range(NCORES)
                )
                if m2 == 0:
                    continue
                grp.append(
                    dict(b=b, phi=phi, jp=jp, m=2 * m2, sigcol=blk["qb"] + 8 * jp)
                )
            grp.sort(key=lambda g: -g["m"])
            chunks.extend(grp)
        assert chunks and chunks[0]["m"] == max(ch["m"] for ch in chunks)
        per_block.append(chunks)

    # kin column offsets in global emission order + slab grouping for DMA
    slabs = []  # (name, col0, width)
    moff = 0
    for b in range(4):
        chunks = per_block[b]
        if b == 0:
            # split each phase group in two for finer DMA pipelining
            groups = []
            for phi in range(NPHASE):
                g = [ch for ch in chunks if ch["phi"] == phi]
                h = (len(g) + 1) // 2
                groups.append((f"b0p{phi}a", g[:h]))
                groups.append((f"b0p{phi}b", g[h:]))
        elif b == 1:
            groups = [
                (f"b1p{phi}", [ch for ch in chunks if ch["phi"] == phi])
                for phi in range(NPHASE)
            ]
        else:
            groups = [(f"b{b}", chunks)]
        for name, g in groups:
            col0 = moff
            for ch in g:
                ch["moff"] = moff
                ch["slab"] = name
                moff += ch["m"]
            slabs.append((name, col0, moff - col0))
    return per_block, slabs, moff


CHUNKS_PER_BLOCK, KIN_SLABS, KIN_COLS = _chunk_schedule()
M0S = [chunks[0]["m"] for chunks in CHUNKS_PER_BLOCK]  # rows written per block
OUT_ROWS = 512  # 4 blocks x 128 rows


# ----------------------------------------------------------------------------
# Host staging
# ----------------------------------------------------------------------------
def _cqt_kernel_bank():
    """Interleaved (re, im) kernel bank, float32 [2*N_BINS, KW].

    Row 2*b is kr[bin b], row 2*b+1 is ki[bin b]. Computed in float64 with
    the reference's formulas (Hann window of per-bin constant-Q length,
    L1-normalized, 1/sqrt(L) scaling).
    """
    kw = max(blk["t0"] + HOP * NCORES * blk["j8"] for blk in BLOCKS)
    bank = np.zeros((2 * N_BINS, kw), dtype=np.float32)
    t = np.arange(kw, dtype=np.float64) - CENTER
    two_pi = 2.0 * np.pi
    step = 36
    for s in range(0, N_BINS, step):
        e = min(s + step, N_BINS)
        freqs = FREQS[s:e, None]
        lens = LENGTHS[s:e, None]
        win = np.where(
            np.abs(t)[None, :] < lens / 2.0,
            0.5 * (1.0 + np.cos(two_pi * t[None, :] / lens)),
            0.0,
        )
        ang = two_pi * freqs * t[None, :] / SR
        norm = win.sum(axis=-1, keepdims=True) * np.sqrt(lens)
        bank[2 * s : 2 * e : 2] = (win * np.cos(ang) / norm).astype(np.float32)
        bank[2 * s + 1 : 2 * e : 2] = (win * np.sin(ang) / norm).astype(np.float32)
    return bank


def _stage_inputs(signal):
    """Build per-core {sig, kin} input maps."""
    signal = np.asarray(signal, dtype=np.float32).reshape(SIG_LEN)
    padded = np.zeros(PAD_LEN, dtype=np.float32)
    padded[CENTER : CENTER + SIG_LEN] = signal
    # SigT[phi][k, i] = padded[HOP*i + KCH*phi + k]
    sig_t = np.ascontiguousarray(padded.reshape(N_SIGCOLS, HOP).T)  # [HOP, 600]

    bank = _cqt_kernel_bank()

    in_maps = []
    for c in range(NCORES):
        sig_c = np.empty((KCH, NPHASE * W_SIG), dtype=np.float32)
        for phi in range(NPHASE):
            sig_c[:, phi * W_SIG : (phi + 1) * W_SIG] = sig_t[
                KCH * phi : KCH * (phi + 1), c : c + W_SIG
            ]
        kin_c = np.zeros((KCH, KIN_COLS), dtype=np.float32)
        for chunks in CHUNKS_PER_BLOCK:
            b = chunks[0]["b"]
            blk = BLOCKS[b]
            for ch in chunks:
                k0 = blk["t0"] + HOP * (8 * ch["jp"] + c) + KCH * ch["phi"]
                r0 = 2 * blk["bin0"]
                kin_c[:, ch["moff"] : ch["moff"] + ch["m"]] = bank[
                    r0 : r0 + ch["m"], k0 : k0 + KCH
                ].T
        in_maps.append({"sig": sig_c, "kin": kin_c})
    return in_maps


# ----------------------------------------------------------------------------
# Device program (identical on all 8 cores)
# ----------------------------------------------------------------------------
@lru_cache(maxsize=1)
def _build_program():
    import concourse.bass as bass  # noqa: F401
    import concourse.mybir as mybir
    from concourse import bacc
    from concourse.tile import TileContext

    f32 = mybir.dt.float32
    f32r = mybir.dt.float32r  # same bits as f32; PE streams 1 col/cycle vs 4
    nc = bacc.Bacc("TRN2", target_bir_lowering=False, debug=False, num_devices=NCORES)
    sig = nc.dram_tensor("sig", [KCH, NPHASE * W_SIG], f32r, kind="ExternalInput")
    kin = nc.dram_tensor("kin", [KCH, KIN_COLS], f32r, kind="ExternalInput")
    out = nc.dram_tensor("out", [OUT_ROWS, N_FRAMES], f32, kind="ExternalOutput")

    slab_cols = {name: (c0, w) for name, c0, w in KIN_SLABS}

    with TileContext(nc) as tc:
        with (
            tc.tile_pool(name="sigp", bufs=1) as sigp,
            tc.tile_pool(name="kinp", bufs=1) as kinp,
            tc.tile_pool(name="psp", bufs=4, space="PSUM") as psp,
            tc.tile_pool(name="outp", bufs=2) as outp,
        ):
            # Issue DMAs in the order the PE consumes the data: each phase's
            # signal window just before block-0's kin slabs of that phase,
            # then the smaller blocks' slabs.
            sig_tiles = [None] * NPHASE

            def load_sig(phi):
                st = sigp.tile([KCH, W_SIG], f32r, tag=f"sig{phi}", name=f"sig{phi}")
                nc.sync.dma_start(st[:], sig[:, phi * W_SIG : (phi + 1) * W_SIG])
                sig_tiles[phi] = st

            slab_tiles = {}

            def load_slab(name):
                c0, w = slab_cols[name]
                t = kinp.tile([KCH, w], f32r, tag=f"kin_{name}", name=f"kin_{name}")
                nc.sync.dma_start(t[:], kin[:, c0 : c0 + w])
                slab_tiles[name] = t

            for phi in range(NPHASE):
                load_sig(phi)
                load_slab(f"b0p{phi}a")
                load_slab(f"b0p{phi}b")
            for phi in range(NPHASE):
                load_slab(f"b1p{phi}")
            load_slab("b2")
            load_slab("b3")

            for b in range(4):
                chunks = CHUNKS_PER_BLOCK[b]
                ps = psp.tile([128, N_FRAMES], f32, tag="ps", name=f"ps{b}")
                for i, ch in enumerate(chunks):
                    kt = slab_tiles[ch["slab"]]
                    c0, _ = slab_cols[ch["slab"]]
                    lo = ch["moff"] - c0
                    nc.tensor.matmul(
                        ps[0 : ch["m"], :],
                        kt[:, lo : lo + ch["m"]],
                        sig_tiles[ch["phi"]][:, ch["sigcol"] : ch["sigcol"] + N_FRAMES],
                        start=(i == 0),
                        stop=(i == len(chunks) - 1),
                    )
                m0 = M0S[b]
                ot = outp.tile([m0, N_FRAMES], f32, tag="out", name=f"out{b}")
                nc.vector.tensor_copy(out=ot[:], in_=ps[0:m0, :])
                nc.sync.dma_start(out[128 * b : 128 * b + m0, :], ot[:])
    nc.compile()
    return nc


# ----------------------------------------------------------------------------
# Entry points
# ----------------------------------------------------------------------------
def _run(signal, trace=False, trace_cores=None):
    from concourse import bass_utils

    nc = _build_program()
    in_maps = _stage_inputs(signal)
    res = bass_utils.run_bass_kernel_spmd(
        nc,
        in_maps,
        core_ids=list(range(NCORES)),
        trace=trace,
        trace_cores=trace_cores,
    )
    total = np.zeros((OUT_ROWS, N_FRAMES), dtype=np.float64)
    for c in range(NCORES):
        total += res.results[c]["out"]
    mag = np.empty((N_BINS, N_FRAMES), dtype=np.float64)
    for b, blk in enumerate(BLOCKS):
        rows = total[128 * b : 128 * b + 2 * blk["nbins"]]
        cr = rows[0::2]
        ci = rows[1::2]
        mag[blk["bin0"] : blk["bin0"] + blk["nbins"]] = np.sqrt(cr * cr + ci * ci)
    return mag.astype(np.float32), res


def kernel(signal):
    out, _ = _run(signal)
    return out
